# revision 22
# baseline (speedup 1.0000x reference)
"""Windowed attention (swin-style, 49-token windows, 8 heads) with DynamicPosBias.

Strategy: data-parallel over B=2048 windows -> 256 windows/core on 8 cores.
Windows are processed in PAIRS (98 partitions). The QK matmul contracts over
128 rows: 64 head-dims + 49 identity rows (fuses the relative-position bias,
host-precomputed from the tiny MLP) + 2 mask rows that push cross-window
scores to -240 so exp(0.125*s) flushes them to zero in fp16. That lets PV
run full-height over both windows with plain (non-block-diagonal) V.
V carries a fused ones-column so PV also emits the softmax denominators.

Perf notes:
- All stationary operands are sliced 128 columns wide (over-reading into the
  neighbouring columns) so the compiler's fast-weight-load kicks in; the junk
  output rows 98:128 are nullified by zeroed pad regions downstream.
- st lives in one 2-bank PSUM tile so exp is a single ACT call per pair
  (heads 0-3 at cols 0:392, heads 4-7 at 512:904 to respect bank bounds).
- K and Q share one SBUF tile (k section | q section) loaded by a single
  large contiguous DMA per block; V likewise (both windows in one DMA).
  Input loads, V loads and output stores ride different DMA rings
  (sync HWDGE / gpsimd SWDGE / scalar HWDGE) so they overlap; 3-deep
  buffering keeps the engines fed across block boundaries.
"""

import numpy as np
from contextlib import ExitStack

import concourse.bass as bass
import concourse.mybir as mybir
import concourse.tile as tile
from concourse import bacc
from concourse.bass_utils import run_bass_kernel_spmd

G = 7
NTOK = 49          # tokens per window
H = 8              # heads
HD = 64            # head dim
C = 512
B = 2048
NCORES = 8
W = B // NCORES    # windows per core = 256
NPAIR = W // 2     # 128
NP = 8             # pairs per block
NBLK = NPAIR // NP # 16
PK = 2 * NTOK      # 98: paired token dim
KROWS = 128        # QK contraction rows: 64 qk + 49 bias + 2 mask + 13 zero
MASKV = -240.0     # cross-window additive mask (scaled by 0.125 -> -30)
FREEK = NP * H * PK  # 6272 cols per k (or q) section
NSLOT = 3
F32 = mybir.dt.float32
F16 = mybir.dt.float16

_CACHED_NC = None
LAST_RESULTS = None


def _rel_idx():
    coords = np.stack(np.meshgrid(np.arange(G), np.arange(G), indexing="ij")).reshape(2, -1)
    rel = (coords[:, :, None] - coords[:, None, :]).transpose(1, 2, 0).copy()
    rel[:, :, 0] += G - 1
    rel[:, :, 1] += G - 1
    rel[:, :, 0] *= 2 * G - 1
    return rel.sum(-1)  # [i, j] in [0, 169)


def _host_pos_mlp(pos_proj_w, pos_proj_b, ln1_g, ln1_b, w1, b1,
                  ln2_g, ln2_b, w2, b2, ln3_g, ln3_b, w3, b3):
    def ln(x, g, b, eps=1e-5):
        mu = x.mean(-1, keepdims=True)
        var = ((x - mu) ** 2).mean(-1, keepdims=True)
        return (x - mu) / np.sqrt(var + eps) * g + b

    pb = np.arange(1 - G, G, dtype=np.float64)
    biases = np.stack(np.meshgrid(pb, pb, indexing="ij")).reshape(2, -1).T  # [169, 2]
    pos = biases @ pos_proj_w + pos_proj_b
    pos = np.maximum(ln(pos, ln1_g, ln1_b), 0) @ w1 + b1
    pos = np.maximum(ln(pos, ln2_g, ln2_b), 0) @ w2 + b2
    pos = np.maximum(ln(pos, ln3_g, ln3_b), 0) @ w3 + b3  # [169, H]
    return pos


# st column offset per head: heads 0-3 in PSUM bank 0, heads 4-7 in bank 1
_STOFF = [512 * (h // 4) + PK * (h % 4) for h in range(H)]


def _build_nc():
    global _CACHED_NC
    if _CACHED_NC is not None:
        return _CACHED_NC
    nc = bacc.Bacc(None, target_bir_lowering=False)

    kqt_d = nc.dram_tensor("kqt", [NBLK, HD, 2 * FREEK + 32], F16, kind="ExternalInput")
    vab_d = nc.dram_tensor("vab", [NBLK, PK, NP * H * 65], F16, kind="ExternalInput")
    kqconst_d = nc.dram_tensor("kqconst", [KROWS - HD, 2 * FREEK + 32], F16, kind="ExternalInput")
    out_d = nc.dram_tensor("out", [NBLK, PK, NP * C], F16, kind="ExternalOutput")

    with tile.TileContext(nc) as tc, ExitStack() as ctx:
        vpool = ctx.enter_context(tc.tile_pool(name="vpool", bufs=NSLOT))
        opool = ctx.enter_context(tc.tile_pool(name="opool", bufs=NSLOT))
        expool = ctx.enter_context(tc.tile_pool(name="expool", bufs=3))
        rpool = ctx.enter_context(tc.tile_pool(name="rpool", bufs=3))
        cpool = ctx.enter_context(tc.tile_pool(name="cpool", bufs=1))
        stps = ctx.enter_context(tc.tile_pool(name="stps", bufs=2, space="PSUM"))
        pvps = ctx.enter_context(tc.tile_pool(name="pvps", bufs=2, space="PSUM"))

        # persistent block buffers: cols [0:FREEK] = K section, [FREEK:2F] = Q
        # section; rows >= HD hold the constant rows (identity+mask / rpb+mask)
        kqblocks = [cpool.tile([KROWS, 2 * FREEK + 32], F16, tag=f"kq{s}", name=f"kq{s}")
                    for s in range(NSLOT)]
        for s in range(NSLOT):
            nc.gpsimd.dma_start(kqblocks[s][HD:KROWS, :], kqconst_d[:])

        for blk in range(NBLK):
            kqb = kqblocks[blk % NSLOT]
            # SWDGE descriptors run ~24-27GB/s vs ~13-16GB/s for HWDGE, but a
            # single SWDGE ring can't feed all engines: split bytes across both
            halfk = FREEK + 16
            nc.gpsimd.dma_start(kqb[0:HD, 0:halfk], kqt_d[blk][:, 0:halfk])
            nc.sync.dma_start(kqb[0:HD, halfk:], kqt_d[blk][:, halfk:])
            vblock = vpool.tile([KROWS, NP * H * 65], F16, tag="v")
            if blk < NSLOT:
                # rows 98:128 must be zero; memset the 32-aligned range 96:128
                # first, the vab DMA below rewrites rows 96:98 with real data
                nc.gpsimd.memset(vblock[96:KROWS, :], 0.0)
            nc.gpsimd.dma_start(vblock[0:PK, :], vab_d[blk])
            oblock = opool.tile([PK, NP * C], F16, tag="o")

            for p in range(NP):
                base = p * H * PK
                st = stps.tile([KROWS, 1024], F32, tag="st", name="st")
                for h in range(H):
                    nc.tensor.matmul(
                        out=st[:, _STOFF[h] : _STOFF[h] + PK],
                        lhsT=kqb[:, base + PK * h : base + PK * h + 128],
                        rhs=kqb[:, FREEK + 32 + base + PK * h : FREEK + 32 + base + PK * (h + 1)],
                        start=True, stop=True,
                    )
                ex = expool.tile([KROWS, H * PK + 32], F16, tag="ex")
                if blk == 0 and p < 3:
                    nc.gpsimd.memset(ex[:, H * PK : H * PK + 32], 0.0)
                nc.scalar.activation(
                    ex[:, 0 : H * PK].rearrange("p (a b) -> p a b", a=2),
                    st[:].rearrange("p (a b) -> p a b", a=2)[:, :, 0 : 4 * PK],
                    mybir.ActivationFunctionType.Exp, scale=0.125,
                )
                # PV: plain full-height V (cross-window ex ~ 0), 4+4 heads per bank
                pv = [pvps.tile([KROWS, 4 * 65], F32, tag=f"pv{t}", name=f"pv{t}")
                      for t in range(2)]
                vbase = p * H * 65
                for h in range(H):
                    t, m = h // 4, h % 4
                    nc.tensor.matmul(
                        out=pv[t][:, 65 * m : 65 * (m + 1)],
                        lhsT=ex[:, PK * h : PK * h + 128],
                        rhs=vblock[:, vbase + 65 * h : vbase + 65 * (h + 1)],
                        start=True, stop=True,
                    )
                rec = rpool.tile([PK, H], F32, tag="rec")
                for t in range(2):
                    nc.vector.reciprocal(
                        rec[:, 4 * t : 4 * t + 4].rearrange("p (h o) -> p h o", o=1),
                        pv[t][0:PK].rearrange("p (h c) -> p h c", c=65)[:, :, 64:65],
                    )
                for t in range(2):
                    nc.vector.tensor_tensor(
                        out=oblock[:, p * C + 4 * t * HD : p * C + (4 * t + 4) * HD].rearrange(
                            "p (h c) -> p h c", c=HD),
                        in0=pv[t][0:PK].rearrange("p (h c) -> p h c", c=65)[:, :, 0:HD],
                        in1=rec[:, 4 * t : 4 * t + 4]
                        .rearrange("p (h o) -> p h o", o=1)
                        .to_broadcast([PK, 4, HD]),
                        op=mybir.AluOpType.mult,
                    )
            half = NP * C // 2
            nc.scalar.dma_start(out_d[blk][:, 0:half], oblock[:, 0:half])
            nc.scalar.dma_start(out_d[blk][:, half:], oblock[:, half:])

    nc.finalize()
    _CACHED_NC = nc
    return nc


def kernel(q, k, v, pos_proj_w, pos_proj_b, ln1_g, ln1_b, w1, b1,
           ln2_g, ln2_b, w2, b2, ln3_g, ln3_b, w3, b3):
    q = np.asarray(q, dtype=np.float32)
    k = np.asarray(k, dtype=np.float32)
    v = np.asarray(v, dtype=np.float32)

    # host-side DynamicPosBias MLP -> rpb[h, i, j] (i=query, j=key), pre-scaled by 8
    pos = _host_pos_mlp(
        np.asarray(pos_proj_w, np.float64), np.asarray(pos_proj_b, np.float64),
        np.asarray(ln1_g, np.float64), np.asarray(ln1_b, np.float64),
        np.asarray(w1, np.float64), np.asarray(b1, np.float64),
        np.asarray(ln2_g, np.float64), np.asarray(ln2_b, np.float64),
        np.asarray(w2, np.float64), np.asarray(b2, np.float64),
        np.asarray(ln3_g, np.float64), np.asarray(ln3_b, np.float64),
        np.asarray(w3, np.float64), np.asarray(b3, np.float64))
    rpb = pos[_rel_idx()]                      # [i, j, h]
    rpb8 = 8.0 * rpb.transpose(1, 2, 0)        # [j, h, i]

    # kconst rows (KROWS-HD = 64): 49 identity + maskA/maskB indicator + zeros
    # layout per head-block of 98 cols: (w in {A,B}) x (49 tokens)
    kc = np.zeros((KROWS - HD, H, 2, NTOK), np.float32)
    kc[0:NTOK, :, :, :] = np.eye(NTOK, dtype=np.float32)[:, None, None, :]
    kc[NTOK, :, 0, :] = 1.0      # row 113: marks w=A key columns
    kc[NTOK + 1, :, 1, :] = 1.0  # row 114: marks w=B key columns
    # qconst rows: rpb8 duplicated across w', then mask values on rows 113/114
    qc = np.zeros((KROWS - HD, H, 2, NTOK), np.float32)
    qc[0:NTOK] = rpb8[:, :, None, :]
    qc[NTOK, :, 1, :] = MASKV    # (A keys) x (B queries) -> -240
    qc[NTOK + 1, :, 0, :] = MASKV
    kqconst = np.concatenate([
        np.tile(kc.reshape(KROWS - HD, H * PK), (1, NP)),
        np.zeros((KROWS - HD, 32), np.float32),
        np.tile(qc.reshape(KROWS - HD, H * PK), (1, NP)),
    ], axis=1).astype(np.float16)

    ones = np.ones((NBLK, NP, 2, NTOK, H, 1), np.float32)
    in_maps = []
    for c in range(NCORES):
        sl = slice(c * W, (c + 1) * W)
        qc_ = q[sl].reshape(NBLK, NP, 2, NTOK, H, HD)
        kc_ = k[sl].reshape(NBLK, NP, 2, NTOK, H, HD)
        vc_ = v[sl].reshape(NBLK, NP, 2, NTOK, H, HD)
        # [blk, d, np, h, w, j] for each of k, q; concatenated as (k | q)
        kt = kc_.transpose(0, 5, 1, 4, 2, 3).reshape(NBLK, HD, FREEK)
        qt = qc_.transpose(0, 5, 1, 4, 2, 3).reshape(NBLK, HD, FREEK)
        kqt = np.ascontiguousarray(
            np.concatenate([kt, np.zeros((NBLK, HD, 32), kt.dtype), qt],
                           axis=2).astype(np.float16))
        vaug = np.concatenate([vc_, ones], axis=5)
        # [blk, (w, j), np, h, c65]
        vab = np.ascontiguousarray(
            vaug.transpose(0, 2, 3, 1, 4, 5).astype(np.float16)
        ).reshape(NBLK, PK, NP * H * 65)
        m = {"kqt": kqt, "vab": vab, "kqconst": kqconst}
        in_maps.append(m)

    nc = _build_nc()
    res = run_bass_kernel_spmd(nc, in_maps, core_ids=list(range(NCORES)))
    global LAST_RESULTS
    LAST_RESULTS = res
    outs = []
    for r in res.results:
        o = r["out"].reshape(NBLK, 2, NTOK, NP, H, HD)   # [blk, w, i, np, h, c]
        o = o.transpose(0, 3, 1, 2, 4, 5).reshape(W, NTOK, C)
        outs.append(o)
    return np.concatenate(outs, axis=0).astype(np.float32)
